# revision 3
# baseline (speedup 1.0000x reference)
"""Two-layer GraphConv (gather + segment-mean + linear + ReLU) x2 + sigmoid head,
distributed over 8 NeuronCores. v4: dst-tile-major stream, per-run gathers with
sentinel trimming, single PSUM chain per tile, tails on the Scalar engine.

Sharding: destination nodes are partitioned across the 8 cores (12.5k each).
Host-side prep: each core's edges are bucketed by (src-chunk-of-25k, dst-tile),
each (chunk x dst-tile) run padded to a multiple of 128; all 8 cores share one
SPMD program sized by the max-core quota. The stream is ordered dst-tile-major:
the 4 chunk-runs of a tile are adjacent, so the whole tile accumulates in one
PSUM matmul chain (no SBUF accumulator, no DVE adds).

Per run (one dma_gather each, 4 SWDGE queues round-robin):
  - gather tables are [N, 128] bf16 (row = 64 feats + 64 pad = 256B descriptor);
    per-core padding positions carry index -1, which the Q7 descriptor
    generator trims, so each core only fetches its own ceil-128 edge rows
  - one-hot built in bf16 on DVE (iota vs drel compare)
  - TensorE bf16 matmuls (lhsT=one-hot, rhs=msgs cols 0:64) accumulate the
    tile's segment-sum across all 4 runs in one PSUM tile
At each tile's last run: Scalar scales by 1/deg (PSUM read), PE transposes,
Scalar copies, PE does W+bias (fp32), Scalar applies ReLU -> bf16 x1 tile,
which is DMA'd out immediately; AllGather (bf16) between layers; layer-2 tail
ends with sigmoid(scale*rowsum+bias).
"""

import os
import sys

for _p in ("/opt/trn_rl_repo", "/opt/pypackages"):
    if _p not in sys.path and os.path.isdir(_p):
        sys.path.insert(0, _p)

import numpy as np

from concourse import bacc, bass, mybir, tile
from concourse.bass_utils import run_bass_kernel_spmd

F32 = mybir.dt.float32
BF16 = mybir.dt.bfloat16
I16 = mybir.dt.int16

TILE = 128


def _cdiv(a, b):
    return (a + b - 1) // b


def balance_labels(edge_src, edge_dst, cfg):
    """Relabel nodes (within each src-chunk) so dst-tile buckets balance.

    The quota structure pads every (tile, chunk) run to the max count over the
    8 cores, then to a multiple of 128. Random tiling leaves ~25% padding. We
    choose which dsts share a (core, tile) bin so that each bin's per-chunk
    edge counts approach the global mean (~510 < 512 = 4*128), collapsing most
    quotas to exactly 4 groups.

    Constraint: node labels only permute WITHIN their 25k chunk (the chunk of
    a gathered source row must stay fixed or the bucketing becomes circular).
    Chunk k's rows map to cores 2k, 2k+1, i.e. 196 bins of the 784.

    Returns new_label [N] such that node v gets gather/output row new_label[v].
    """
    N, CH, NP, NT, C = cfg.N, cfg.CH, cfg.NP, cfg.NT, cfg.C
    NDST = cfg.NDST
    src = np.asarray(edge_src).astype(np.int64)
    dst = np.asarray(edge_dst).astype(np.int64)
    p = src // CH
    degv = np.zeros((N, NP), np.float64)
    np.add.at(degv, (dst, p), 1.0)

    # The mean (tile, chunk) bucket load is exactly 128*avgdeg/NP (~512 here),
    # sitting on the ceil-128 cliff. Tiered caps: a few overflow tiles take a
    # 640 quota so every other bucket can stay <= 512 on all cores.
    NOVF = 6  # overflow tiles (shared across all chunks)
    MARGIN = 3.0

    new_label = np.empty(N, np.int64)
    for k in range(NP):
        nodes = np.arange(k * CH, (k + 1) * CH)
        v = degv[nodes]
        # bins: (core, tile) for core in {2k, 2k+1}; REM tile (t=NT-1) cap 84
        bins = [(2 * k + ci, t) for ci in range(2) for t in range(NT)]
        lane_cap = np.array([NDST - (NT - 1) * TILE if t == NT - 1 else TILE
                             for (_, t) in bins], np.int64)
        nb = len(bins)

        def bucket_cap(t):
            if t == NT - 1:
                return 384.0
            return 640.0 if t < NOVF else 512.0

        caps = np.array([[bucket_cap(t)] * NP for (_, t) in bins], np.float64)
        caps -= MARGIN
        bin_sum = np.zeros((nb, NP))
        bin_cnt = np.zeros(nb, np.int64)
        members = [[] for _ in range(nb)]
        tot = v.sum(1)
        heavy = np.argsort(-tot, kind="stable")
        heavy = heavy[tot[heavy] > 0]
        for d in heavy:
            vd = v[d]
            avail = np.nonzero(bin_cnt < lane_cap)[0]
            over = (bin_sum[avail] + vd[None, :]) - caps[avail]
            s = over.max(1) + 0.001 * (bin_cnt[avail] / lane_cap[avail])
            b = avail[np.argmin(s)]
            bin_sum[b] += vd
            members[b].append(d)
            bin_cnt[b] += 1
        # fill zero-degree nodes into remaining slots
        zeros = np.nonzero(tot == 0)[0]
        zi = 0
        for b in range(nb):
            room = lane_cap[b] - bin_cnt[b]
            if room > 0:
                for d in zeros[zi:zi + room]:
                    members[b].append(d)
                zi += room
        assert zi == len(zeros)

        # repair: swap nodes between bins until per-bucket caps hold
        hard = caps + MARGIN  # the true ceil-128 boundaries
        assign = np.empty(CH, np.int64)
        for b in range(nb):
            assign[np.array(members[b], np.int64)] = b
        for _ in range(60):
            over = bin_sum - hard
            viol = np.argwhere(over > 0)
            if len(viol) == 0:
                break
            viol = viol[np.argsort(-over[viol[:, 0], viol[:, 1]])]
            progress = False
            for b, pp in viol:
                if bin_sum[b, pp] <= hard[b, pp]:
                    continue
                mem_b = np.nonzero(assign == b)[0]
                donors = mem_b[np.argsort(-v[mem_b, pp])]
                done = False
                for d in donors[:6]:
                    if v[d, pp] == 0:
                        break
                    slack = hard - bin_sum  # [nb, NP]
                    fits = np.all(slack - v[d][None, :] >= 0, axis=1)
                    fits[b] = False
                    cand = np.nonzero(fits)[0]
                    if len(cand) == 0:
                        continue
                    # prefer recipients with most slack in chunk pp
                    cand = cand[np.argsort(-slack[cand, pp])][:8]
                    for b2 in cand:
                        mem_b2 = np.nonzero(assign == b2)[0]
                        # swap-back node: minimal impact on b at chunk pp
                        sb = np.argsort(v[mem_b2, pp] * 100 + v[mem_b2].sum(1))
                        for d2 in mem_b2[sb[:4]]:
                            nb_sum = bin_sum[b] - v[d] + v[d2]
                            nb2_sum = bin_sum[b2] + v[d] - v[d2]
                            if (np.all(nb2_sum <= hard[b2])
                                    and nb_sum[pp] < bin_sum[b, pp]
                                    and np.all(nb_sum <= np.maximum(
                                        hard[b], bin_sum[b]))):
                                bin_sum[b] = nb_sum
                                bin_sum[b2] = nb2_sum
                                assign[d], assign[d2] = b2, b
                                progress = True
                                done = True
                                break
                        if done:
                            break
                    if done:
                        break
            if not progress:
                break
        members = [[] for _ in range(nb)]
        for d in range(CH):
            members[assign[d]].append(d)
        for b, (c, t) in enumerate(bins):
            mem = np.array(members[b], np.int64)
            rows = c * NDST + t * TILE + np.arange(len(mem))
            new_label[nodes[mem]] = rows
    return new_label


class Cfg:
    def __init__(self, N=100000, D=64, C=8, CH=25000, BSZ=1024, no_cc=False):
        self.no_cc = no_cc
        assert N % C == 0 and N % CH == 0
        assert CH <= 32768  # int16 gather indices
        self.N, self.D, self.C, self.CH = N, D, C, CH
        self.NDST = N // C
        self.NT = _cdiv(self.NDST, TILE)
        self.NP = N // CH
        self.D2 = 32  # layer-2 output width


def plan_edges(edge_src, edge_dst, cfg):
    """Bucket/sort/pad edges per core; all cores share the quota structure.

    Stream is dst-tile-major: runs ordered (t, p)."""
    src = np.asarray(edge_src).astype(np.int64)
    dst = np.asarray(edge_dst).astype(np.int64)
    C, CH, NT, NP, NDST = cfg.C, cfg.CH, cfg.NT, cfg.NP, cfg.NDST

    percore = []
    counts = []
    for c in range(C):
        m = (dst // NDST) == c
        s = src[m]
        dl = dst[m] - c * NDST
        p = s // CH
        t = dl >> 7
        o = np.lexsort((dl, p, t))  # t-major, then chunk, then dst
        s, dl, p, t = s[o], dl[o], p[o], t[o]
        cnt = np.bincount(t * NP + p, minlength=NT * NP).reshape(NT, NP)
        percore.append((s, dl, p, t))
        counts.append(cnt)

    quota = np.maximum.reduce(counts)  # [NT, NP]
    quota = (quota + TILE - 1) // TILE * TILE
    qflat = quota.reshape(-1)
    offs = np.concatenate([[0], np.cumsum(qflat)])
    T = int(offs[-1])
    offs_flat = offs[:-1].reshape(NT, NP)

    # runs in stream order, skipping empty ones
    runs = []  # (t, p, q, boff, first_of_tile, last_of_tile)
    for t in range(NT):
        nz = [p for p in range(NP) if quota[t, p] > 0]
        assert nz, f"tile {t} has no edges on any core"
        for p in nz:
            runs.append((t, p, int(quota[t, p]), int(offs_flat[t, p]),
                         p == nz[0], p == nz[-1]))

    per_core_arrays = []
    for c in range(C):
        s, dl, p, t = percore[c]
        key = t * NP + p
        first = np.searchsorted(key, np.arange(NT * NP), side="left")
        rank = np.arange(len(key)) - first[key]
        pos = offs_flat[t, p] + rank
        srcl = np.zeros(T, np.int16)  # padding gathers row 0 (real, finite data)
        drel = np.full(T, 200.0, np.float32)  # sentinel: never matches iota
        srcl[pos] = (s - p * CH).astype(np.int16)
        drel[pos] = (dl - (t << 7)).astype(np.float32)

        deg = np.bincount(dl, minlength=NDST).astype(np.float32)
        deg = np.maximum(deg, 1.0)
        degp = np.ones(NT * TILE, np.float32)
        degp[:NDST] = deg
        deg_arr = degp.reshape(NT, TILE).T.copy()  # [128, NT]

        idxw = np.tile(srcl.reshape(T // 16, 16).T, (8, 1)).copy()  # [128, T/16]
        drw = drel.reshape(T // TILE, TILE).T.copy()  # [128, T/128]
        per_core_arrays.append(dict(idxs=idxw, drel=drw, deg=deg_arr))

    structure = dict(T=T, MAXQ=int(quota.max()), runs=tuple(runs))
    return structure, per_core_arrays


def build_program(cfg, structure):
    N, D, C, CH, NT, NP = cfg.N, cfg.D, cfg.C, cfg.CH, cfg.NT, cfg.NP
    D2 = cfg.D2
    NDST = cfg.NDST
    T = structure["T"]
    MAXQ = structure["MAXQ"]
    runs = structure["runs"]
    Relu = mybir.ActivationFunctionType.Relu
    Copy = mybir.ActivationFunctionType.Copy
    Sigmoid = mybir.ActivationFunctionType.Sigmoid

    RW = 128  # bf16 table row width: 64 feats + 64 pad = 256B descriptors

    nc = bacc.Bacc(None, target_bir_lowering=False, num_swdge_queues=4)
    x0bf = nc.dram_tensor("x0bf", [N, RW], BF16, kind="ExternalInput")
    idxs_d = nc.dram_tensor("idxs", [128, T // 16], I16, kind="ExternalInput")
    drel_d = nc.dram_tensor("drel", [128, T // TILE], F32, kind="ExternalInput")
    deg_d = nc.dram_tensor("deg", [128, NT], F32, kind="ExternalInput")
    w1_d = nc.dram_tensor("w1", [D, D], F32, kind="ExternalInput")
    b1_d = nc.dram_tensor("b1", [1, D], F32, kind="ExternalInput")
    w2_d = nc.dram_tensor("w2", [D, D2], F32, kind="ExternalInput")
    b2_d = nc.dram_tensor("b2", [1, D2], F32, kind="ExternalInput")
    wdbd_d = nc.dram_tensor("wdbd", [1, 2], F32, kind="ExternalInput")
    iota_d = nc.dram_tensor("iota", [128, MAXQ], F32, kind="ExternalInput")
    ident_d = nc.dram_tensor("ident", [128, 128], F32, kind="ExternalInput")
    ones_d = nc.dram_tensor("ones1", [1, 128], F32, kind="ExternalInput")
    outp = nc.dram_tensor("out", [NDST, 1], F32, kind="ExternalOutput")
    x1loc = nc.dram_tensor("x1loc", [NDST, RW], BF16)
    x1full = nc.dram_tensor("x1full", [N, RW], BF16, addr_space="Shared")

    NFULL = NDST // TILE  # full dst tiles
    REM = NDST - NFULL * TILE  # lanes in the last (partial) tile, 0 if none

    with tile.TileContext(nc) as tc:
        with (
            tc.tile_pool(name="const", bufs=1) as cp,
            tc.tile_pool(name="msgs", bufs=16) as mp,
            tc.tile_pool(name="ohp", bufs=8) as ohp,
            tc.tile_pool(name="tails", bufs=4) as wp,
            tc.tile_pool(name="psacc", bufs=4, space="PSUM") as ps_acc,
            tc.tile_pool(name="pst", bufs=2, space="PSUM") as ps_t,
            tc.tile_pool(name="psm", bufs=2, space="PSUM") as ps_m,
        ):
            # ---- constants into SBUF ----
            iota_f = cp.tile([128, MAXQ], F32)
            nc.sync.dma_start(iota_f[:], iota_d[:, :])
            iota_bf = cp.tile([128, MAXQ], BF16)
            nc.vector.tensor_copy(iota_bf[:], iota_f[:])
            ident_sb = cp.tile([128, 128], F32)
            nc.sync.dma_start(ident_sb[:], ident_d[:, :])
            ones_sb = cp.tile([1, 128], F32)
            nc.sync.dma_start(ones_sb[:], ones_d[:, :])
            w1_sb = cp.tile([D, D], F32)
            nc.sync.dma_start(w1_sb[:], w1_d[:, :])
            b1_sb = cp.tile([1, D], F32)
            nc.sync.dma_start(b1_sb[:], b1_d[:, :])
            w2_sb = cp.tile([D, D2], F32)
            nc.sync.dma_start(w2_sb[:], w2_d[:, :])
            b2_sb = cp.tile([1, D2], F32)
            nc.sync.dma_start(b2_sb[:], b2_d[:, :])
            wdbd_sb = cp.tile([1, 2], F32)
            nc.sync.dma_start(wdbd_sb[:], wdbd_d[:, :])
            deg_sb = cp.tile([128, NT], F32)
            nc.sync.dma_start(deg_sb[:], deg_d[:, :])
            rdeg = cp.tile([128, NT], F32)
            nc.vector.reciprocal(rdeg[:], deg_sb[:])

            # whole-stream edge metadata, shared by both layers
            idx_all = cp.tile([128, T // 16], I16)
            NSPLIT = 4
            step = _cdiv(T // 16, NSPLIT * 8) * 8
            for i in range(NSPLIT):
                c0, c1 = i * step, min((i + 1) * step, T // 16)
                if c0 < c1:
                    nc.sync.dma_start(idx_all[:, c0:c1], idxs_d[:, c0:c1])
            drel_f = cp.tile([128, T // TILE], F32)
            nc.sync.dma_start(drel_f[:], drel_d[:, :])
            drel_bf = cp.tile([128, T // TILE], BF16)
            nc.vector.tensor_copy(drel_bf[:], drel_f[:])

            # broadcast Wd/32 and bd across partitions via a K=1 matmul
            wb_ps = ps_m.tile([128, 64], F32, tag="mm", name="wb_ps")
            nc.tensor.matmul(wb_ps[:, :2], lhsT=ones_sb[:], rhs=wdbd_sb[:],
                             start=True, stop=True)
            wb_rep = cp.tile([128, 2], F32)
            nc.vector.tensor_copy(wb_rep[:], wb_ps[:, :2])
            nc.vector.tensor_scalar_mul(wb_rep[:, 0:1], wb_rep[:, 0:1], 1.0 / 32.0)

            x1sb = cp.tile([128, NT * D], BF16)
            res = cp.tile([128, NT], F32)

            def do_tail(t, ps, last):
                scaled = wp.tile([128, D], F32, tag="scaled")
                nc.scalar.activation(scaled[:], ps[:], Copy,
                                     scale=rdeg[:, t:t + 1])
                tps = ps_t.tile([D, 128], F32, tag="tps")
                nc.tensor.transpose(tps[:], scaled[:], ident_sb[:])
                aggT = wp.tile([D, 128], F32, tag="aggT")
                nc.scalar.activation(aggT[:], tps[:], Copy)
                if not last:
                    x1ps = ps_m.tile([128, D], F32, tag="mm", name="x1ps")
                    nc.tensor.matmul(x1ps[:], lhsT=aggT[:], rhs=w1_sb[:],
                                     start=True, stop=False)
                    nc.tensor.matmul(x1ps[:], lhsT=ones_sb[:], rhs=b1_sb[:],
                                     start=False, stop=True)
                    nc.scalar.activation(
                        x1sb[:, t * D:(t + 1) * D], x1ps[:], Relu)
                    # stream this tile of x1 out to DRAM right away
                    if t < NFULL:
                        nc.sync.dma_start(
                            x1loc[t * TILE:(t + 1) * TILE, :D],
                            x1sb[:, t * D:(t + 1) * D])
                    else:
                        nc.sync.dma_start(
                            x1loc[NFULL * TILE:, :D],
                            x1sb[:REM, t * D:(t + 1) * D])
                else:
                    x2ps = ps_m.tile([128, D], F32, tag="mm", name="x2ps")
                    nc.tensor.matmul(x2ps[:, :D2], lhsT=aggT[:], rhs=w2_sb[:],
                                     start=True, stop=False)
                    nc.tensor.matmul(x2ps[:, :D2], lhsT=ones_sb[:], rhs=b2_sb[:],
                                     start=False, stop=True)
                    x2sb = wp.tile([128, D2], F32, tag="x2sb")
                    ssb = wp.tile([128, 1], F32, tag="ssb")
                    nc.scalar.activation(x2sb[:], x2ps[:, :D2], Relu,
                                         accum_out=ssb[:])
                    nc.scalar.activation(
                        res[:, t:t + 1], ssb[:], Sigmoid,
                        bias=wb_rep[:, 1:2], scale=wb_rep[:, 0:1])

            def do_layer(table, last):
                cur_ps = [None]
                qc = [0]
                for ri, (t, p, q, boff, fot, lot) in enumerate(runs):
                    ncol = q // TILE
                    msgs = mp.tile([128, (MAXQ // TILE) * RW], BF16, tag="msgs")
                    # dma_gather beyond 1024 rows overflows the per-DMA
                    # descriptor ring and wedges the device; split large runs
                    off = 0
                    while off < q:
                        nq = min(1024, q - off)
                        msgs3 = msgs[:, (off // TILE) * RW:
                                     ((off + nq) // TILE) * RW].rearrange(
                            "p (c f) -> p c f", f=RW)
                        nc.gpsimd.dma_gather(
                            msgs3,
                            table[p * CH:(p + 1) * CH, :],
                            idx_all[:, (boff + off) // 16:
                                    (boff + off + nq) // 16],
                            nq,
                            nq,
                            RW,
                            queue_num=qc[0] % 4,
                        )
                        qc[0] += 1
                        off += nq
                    oh = ohp.tile([128, MAXQ], BF16, tag="oh")
                    in1 = (
                        drel_bf[:, boff // TILE:boff // TILE + ncol]
                        .rearrange("p (g o) -> p g o", o=1)
                        .to_broadcast([128, ncol, TILE])
                    )
                    nc.vector.tensor_tensor(
                        out=oh[:, :ncol * TILE],
                        in0=iota_bf[:, :ncol * TILE],
                        in1=in1,
                        op=mybir.AluOpType.is_equal,
                    )
                    if fot:
                        cur_ps[0] = ps_acc.tile(
                            [128, D], F32, tag="acc", name="accps")
                    for g in range(ncol):
                        nc.tensor.matmul(
                            cur_ps[0][:],
                            lhsT=oh[:, g * TILE:(g + 1) * TILE],
                            rhs=msgs[:, g * RW:g * RW + D],
                            start=(fot and g == 0),
                            stop=(lot and g == ncol - 1),
                        )
                    if lot:
                        do_tail(t, cur_ps[0], last)

            # ---------------- layer 1 ----------------
            do_layer(x0bf, last=False)

            if cfg.no_cc:
                nc.sync.dma_start(x1full[:NDST, :], x1loc[:, :])
            else:
                nc.gpsimd.collective_compute(
                    "AllGather",
                    mybir.AluOpType.bypass,
                    replica_groups=[list(range(C))],
                    ins=[x1loc[:, :]],
                    outs=[x1full[:, :]],
                )

            # ---------------- layer 2 + head ----------------
            do_layer(x1full, last=True)

            if NFULL:
                nc.sync.dma_start(
                    outp[: NFULL * TILE, :].rearrange("(t r) o -> r (t o)", r=TILE),
                    res[:, :NFULL],
                )
            if REM:
                nc.sync.dma_start(
                    outp[NFULL * TILE:, :],
                    res[:REM, NFULL:NFULL + 1],
                )

    nc.finalize()
    return nc


_CACHE = {}


def _get_program(cfg, structure):
    key = (cfg.N, cfg.D, cfg.C, cfg.CH, cfg.no_cc,
           structure["T"], structure["MAXQ"], structure["runs"])
    if key not in _CACHE:
        _CACHE[key] = build_program(cfg, structure)
    return _CACHE[key]


# exposed for test.py to rerun with tracing without rebuilding
LAST_RUN = {}


def kernel(node_features, edge_src, edge_dst, W1, b1, W2, b2, Wd, bd,
           cfg=None, trace=False):
    cfg = cfg or Cfg(N=node_features.shape[0])
    new_label = balance_labels(edge_src, edge_dst, cfg)
    src2 = new_label[np.asarray(edge_src).astype(np.int64)]
    dst2 = new_label[np.asarray(edge_dst).astype(np.int64)]
    structure, per_core = plan_edges(src2, dst2, cfg)
    nc = _get_program(cfg, structure)

    import ml_dtypes
    x0 = np.asarray(node_features, dtype=np.float32)
    x0bf = np.zeros((cfg.N, 128), dtype=ml_dtypes.bfloat16)
    x0bf[new_label, :cfg.D] = x0.astype(ml_dtypes.bfloat16)
    iota = np.tile(np.arange(128, dtype=np.float32),
                   structure["MAXQ"] // TILE)[None, :].repeat(128, axis=0).copy()
    ident = np.eye(128, dtype=np.float32)
    ones1 = np.ones((1, 128), np.float32)
    wdbd = np.array([[np.asarray(Wd).reshape(-1)[0],
                      np.asarray(bd).reshape(-1)[0]]], np.float32)
    shared = dict(
        x0bf=x0bf,
        w1=np.ascontiguousarray(np.asarray(W1, np.float32)),
        b1=np.asarray(b1, np.float32).reshape(1, -1),
        w2=np.ascontiguousarray(np.asarray(W2, np.float32)),
        b2=np.asarray(b2, np.float32).reshape(1, -1),
        wdbd=wdbd,
        iota=iota,
        ident=ident,
        ones1=ones1,
    )
    in_maps = []
    for c in range(cfg.C):
        m = dict(shared)
        m.update(per_core[c])
        in_maps.append(m)

    core_ids = list(range(cfg.C))
    r = run_bass_kernel_spmd(nc, in_maps, core_ids, trace=trace)
    LAST_RUN["nc"] = nc
    LAST_RUN["in_maps"] = in_maps
    LAST_RUN["results"] = r
    out = np.concatenate([r.results[c]["out"] for c in range(cfg.C)], axis=0)
    return out[new_label]
